# revision 4
# baseline (speedup 1.0000x reference)
"""Trainium2 Bass kernel for nn_Conv2DSpatial (4-direction recurrent conv).

Math: for each direction d with 1-pixel shift s_d and 64x64 weight W_d:
    t_k = relu(shift(t_{k-1}) @ W_d), t_0 = x;  out_d = x + sum_{k=1..8} t_k
Key identity: t_k[p] = u_k[p - k*s_d] where u_k = f_d^k(x) is a *pointwise*
recurrence (no shifts). So we iterate u_k in place and apply all shifts in a
final 8-term shifted add-tree.

Distribution: data-parallel over batch (8 images -> 8 NeuronCores).
Per core, directions are packed in pairs as 128-wide block-diagonal matmuls:
  pair A = (left, right-mirrored-in-w), layout [c, h, w] (w = inner)
  pair B = (up, down-mirrored-in-h),    layout [c, w, h] (h = inner)
Mirroring/transposition is done on host (numpy) so on-device both pairs run
the identical code: shift axis is always the inner dim, shift is always -k.

Per stripe of S=16 outer rows: 9 bf16 planes [128, 9, S, 8+192] (8 zero guard
cols absorb the negative shifts), 8 matmul+relu steps (relu split between
ScalarE and VectorE), then a 7-add bf16 tree + fp32 finalize with exact fp32 x.
"""

import time

import numpy as np
import ml_dtypes

BF16 = ml_dtypes.bfloat16

B, H, W, C = 8, 192, 192, 64
R = 8           # recurrence steps
S = 16          # stripe rows (outer dim)
G = 8           # guard columns (must cover max shift = R)
WID = G + W     # padded inner width = 200
ACOL = 168      # relu cols done on ScalarE; rest (192-ACOL) on VectorE
NCORES = 8

_CACHE = {}

LAST_EXEC_TIME_NS = None


def _build_module():
    import concourse.bacc as bacc
    import concourse.tile as tile
    from concourse import mybir
    from contextlib import ExitStack

    BF = mybir.dt.bfloat16
    F32 = mybir.dt.float32

    nc = bacc.Bacc("TRN2", target_bir_lowering=False, debug=False,
                   num_devices=NCORES)

    ins = {}
    for p in ("a", "b"):
        for o in ("n", "m"):
            ins[f"x{p}{o}"] = nc.dram_tensor(f"x{p}{o}", [C, H, W], BF,
                                             kind="ExternalInput")
            ins[f"x{p}{o}f"] = nc.dram_tensor(f"x{p}{o}f", [C, H, W], F32,
                                              kind="ExternalInput")
        ins[f"w{p}"] = nc.dram_tensor(f"w{p}", [128, 128], BF,
                                      kind="ExternalInput")
    outs = {p: nc.dram_tensor(f"o{p}", [128, H, W], F32,
                              kind="ExternalOutput") for p in ("a", "b")}

    NS = H // S

    with tile.TileContext(nc) as tc:
        with ExitStack() as ctx:
            planes_pool = ctx.enter_context(tc.tile_pool(name="planes", bufs=2))
            xf_pool = ctx.enter_context(tc.tile_pool(name="xf", bufs=2))
            out_pool = ctx.enter_context(tc.tile_pool(name="out", bufs=2))
            w_pool = ctx.enter_context(tc.tile_pool(name="w", bufs=1))
            psum_pool = ctx.enter_context(
                tc.tile_pool(name="ps", bufs=2, space="PSUM"))

            for p in ("a", "b"):
                wt = w_pool.tile([128, 128], BF)
                nc.sync.dma_start(wt[:], ins[f"w{p}"][:])
                xn, xm = ins[f"x{p}n"], ins[f"x{p}m"]
                xnf, xmf = ins[f"x{p}nf"], ins[f"x{p}mf"]
                od = outs[p]
                for s in range(NS):
                    r0 = s * S
                    T = planes_pool.tile([128, R + 1, S, WID], BF)
                    xf = xf_pool.tile([128, S, W], F32)
                    # zero guard cols of all planes (kills negative-shift taps)
                    nc.vector.memset(T[:, :, :, 0:G], 0.0)
                    # plane 0 = bf16(x); both halves (normal / mirrored)
                    nc.sync.dma_start(T[0:64, 0, :, G:WID], xn[:, r0:r0 + S, :])
                    nc.sync.dma_start(T[64:128, 0, :, G:WID], xm[:, r0:r0 + S, :])
                    # exact fp32 x for the final accumulation
                    nc.sync.dma_start(xf[0:64], xnf[:, r0:r0 + S, :])
                    nc.sync.dma_start(xf[64:128], xmf[:, r0:r0 + S, :])
                    for k in range(1, R + 1):
                        for h in range(0, S, 8):
                            # PSUM chunk: 8 rows, padded 256-stride so each
                            # 2-row matmul output sits in one 2KB bank.
                            Y = psum_pool.tile([128, 8, 256], F32)
                            for g in range(4):
                                rows = slice(h + 2 * g, h + 2 * g + 2)
                                nc.tensor.matmul(
                                    Y[:, 2 * g:2 * g + 2, 0:W],
                                    wt[:],
                                    T[:, k - 1, rows, G:WID],
                                    start=True, stop=True)
                            # relu PSUM->SBUF, split ScalarE / VectorE
                            nc.scalar.activation(
                                T[:, k, h:h + 8, G:G + ACOL],
                                Y[:, :, 0:ACOL],
                                mybir.ActivationFunctionType.Relu)
                            nc.vector.tensor_scalar_max(
                                T[:, k, h:h + 8, G + ACOL:WID],
                                Y[:, :, ACOL:W], 0.0)

                    # shifted add tree: out = x + sum_k u_k[j - k]
                    def Rd(kk, d):
                        return T[:, kk, :, G - d:WID - d]

                    def Wr(kk):
                        return T[:, kk, :, G:WID]

                    va = nc.vector
                    va.tensor_add(Wr(2), Rd(2, 0), Rd(4, 2))   # u2+u4[-2]
                    va.tensor_add(Wr(6), Rd(6, 0), Rd(8, 2))   # u6+u8[-2]
                    va.tensor_add(Wr(2), Rd(2, 0), Rd(6, 4))   # evens rel u2
                    va.tensor_add(Wr(1), Rd(1, 0), Rd(3, 2))   # u1+u3[-2]
                    va.tensor_add(Wr(5), Rd(5, 0), Rd(7, 2))   # u5+u7[-2]
                    va.tensor_add(Wr(1), Rd(1, 0), Rd(5, 4))   # odds rel u1
                    va.tensor_add(Wr(1), Rd(1, 0), Rd(2, 1))   # all 8 rel u1
                    ot = out_pool.tile([128, S, W], F32)
                    va.tensor_add(ot[:], xf[:], Rd(1, 1))      # + exact x
                    nc.sync.dma_start(od[:, r0:r0 + S, :], ot[:])
    nc.finalize()
    return nc


def _ensure_exec():
    """Build the module once and wrap it in a cached jitted SPMD callable."""
    if "sharded" in _CACHE:
        return
    import jax
    from jax.sharding import Mesh, PartitionSpec
    from jax.experimental.shard_map import shard_map
    from concourse import mybir, bass2jax

    bass2jax.install_neuronx_cc_hook()
    nc = _build_module()

    pid_name = (nc.partition_id_tensor.name
                if nc.partition_id_tensor is not None else None)
    in_names, out_names, out_avals = [], [], []
    for alloc in nc.m.functions[0].allocations:
        if not isinstance(alloc, mybir.MemoryLocationSet):
            continue
        name = alloc.memorylocations[0].name
        if alloc.kind == "ExternalInput":
            if name != pid_name:
                in_names.append(name)
        elif alloc.kind == "ExternalOutput":
            out_names.append(name)
            out_avals.append(jax.core.ShapedArray(
                tuple(alloc.tensor_shape), mybir.dt.np(alloc.dtype)))
    n_params = len(in_names)
    all_names = in_names + out_names
    if pid_name is not None:
        all_names = all_names + [pid_name]
    donate = tuple(range(n_params, n_params + len(out_names)))

    def _body(*args):
        operands = list(args)
        if pid_name is not None:
            operands.append(bass2jax.partition_id_tensor())
        outs = bass2jax._bass_exec_p.bind(
            *operands,
            out_avals=tuple(out_avals),
            in_names=tuple(all_names),
            out_names=tuple(out_names),
            lowering_input_output_aliases=(),
            sim_require_finite=True,
            sim_require_nnan=True,
            nc=nc,
        )
        return tuple(outs)

    devices = jax.devices()[:NCORES]
    mesh = Mesh(np.asarray(devices), ("core",))
    nio = n_params + len(out_names)
    sharded = jax.jit(
        shard_map(_body, mesh=mesh,
                  in_specs=(PartitionSpec("core"),) * nio,
                  out_specs=(PartitionSpec("core"),) * len(out_names),
                  check_rep=False),
        donate_argnums=donate, keep_unused=True)

    _CACHE.update(nc=nc, sharded=sharded, mesh=mesh, in_names=in_names,
                  out_names=out_names, out_avals=out_avals)


def _prep_inputs(x, W_left, W_right, W_up, W_down):
    """Host-side layout prep. Returns per-core input maps."""
    wa = np.zeros((128, 128), np.float32)
    wa[0:64, 0:64] = W_left
    wa[64:128, 64:128] = W_right
    wb = np.zeros((128, 128), np.float32)
    wb[0:64, 0:64] = W_up
    wb[64:128, 64:128] = W_down
    wa = wa.astype(BF16)
    wb = wb.astype(BF16)

    in_maps = []
    for b in range(B):
        xb = np.asarray(x[b], np.float32)               # (h, w, c)
        xan = np.ascontiguousarray(xb.transpose(2, 0, 1))              # [c,h,w]
        xam = np.ascontiguousarray(xb[:, ::-1, :].transpose(2, 0, 1))  # w-mir
        xbn = np.ascontiguousarray(xb.transpose(2, 1, 0))              # [c,w,h]
        xbm = np.ascontiguousarray(xb[::-1, :, :].transpose(2, 1, 0))  # h-mir
        in_maps.append({
            "xan": xan.astype(BF16), "xanf": xan,
            "xam": xam.astype(BF16), "xamf": xam,
            "xbn": xbn.astype(BF16), "xbnf": xbn,
            "xbm": xbm.astype(BF16), "xbmf": xbm,
            "wa": wa, "wb": wb,
        })
    return in_maps


def _concat_inputs(in_maps):
    return [np.concatenate([m[name] for m in in_maps], axis=0)
            for name in _CACHE["in_names"]]


def _zero_outs():
    return [np.zeros((NCORES * a.shape[0], *a.shape[1:]), a.dtype)
            for a in _CACHE["out_avals"]]


def _run(concat_in):
    out_arrs = _CACHE["sharded"](*concat_in, *_zero_outs())
    out_avals, out_names = _CACHE["out_avals"], _CACHE["out_names"]
    return [
        {name: np.asarray(out_arrs[i]).reshape(NCORES, *out_avals[i].shape)[c]
         for i, name in enumerate(out_names)}
        for c in range(NCORES)
    ]


def kernel(x, W_left, W_right, W_up, W_down):
    _ensure_exec()
    in_maps = _prep_inputs(np.asarray(x), np.asarray(W_left),
                           np.asarray(W_right), np.asarray(W_up),
                           np.asarray(W_down))
    results = _run(_concat_inputs(in_maps))

    out = np.empty((B, H, W, 4 * C), np.float32)
    for b in range(B):
        oa = results[b]["oa"]   # [128, h, w]
        ob = results[b]["ob"]   # [128, w, h]
        out[b, :, :, 0:64] = oa[0:64].transpose(1, 2, 0)                # left
        out[b, :, :, 64:128] = oa[64:128, :, ::-1].transpose(1, 2, 0)   # right
        out[b, :, :, 128:192] = ob[0:64].transpose(2, 1, 0)             # up
        out[b, :, :, 192:256] = ob[64:128, :, ::-1].transpose(2, 1, 0)  # down
    return out


def bench(in_maps=None, iters=5):
    """Time pure device execution (inputs pre-placed, min over iters)."""
    import jax
    from jax.sharding import NamedSharding, PartitionSpec
    global LAST_EXEC_TIME_NS
    _ensure_exec()
    if in_maps is None:
        rng = np.random.default_rng(0)
        x = rng.standard_normal((B, H, W, C), dtype=np.float32)
        w = [rng.standard_normal((C, C), dtype=np.float32) * 0.05
             for _ in range(4)]
        in_maps = _prep_inputs(x, *w)
    sharding = NamedSharding(_CACHE["mesh"], PartitionSpec("core"))
    dev_in = [jax.device_put(a, sharding) for a in _concat_inputs(in_maps)]
    times = []
    for _ in range(iters):
        zeros = [jax.device_put(z, sharding) for z in _zero_outs()]
        jax.block_until_ready(zeros)
        t0 = time.perf_counter_ns()
        outs = _CACHE["sharded"](*dev_in, *zeros)
        jax.block_until_ready(outs)
        times.append(time.perf_counter_ns() - t0)
    LAST_EXEC_TIME_NS = min(times)
    return times
